# revision 1
# baseline (speedup 1.0000x reference)
"""Trainium2 Bass kernel for nn_Kernel_55722905698800 (gnn_message_passing).

Per edge e (E=20000) the reference builds a 64x64 matrix
  out[e] = sum_p norm_p * einsum('oi,f,abf->(o a)(i b)', Rw_p[e], Y_lf(u_e), W3J_p)
with Rw[e] = silu(gauss_basis(|r_e|) @ W1 + b1) @ W2 + b2 reshaped [6,16,16],
falling back to a constant block-diagonal matrix where |r_e| == 0.

Strategy (8 cores, data parallel over edges; 2560 padded edges/core = 20 tiles
x 128 partitions; edge <-> (partition p, tile t) = p*20+t):
  - All path norms / Wigner-3j constants fold into W2/b2 host-side, so each
    output block is a per-edge-scalar x 16x16-radial-block product:
      block00            = Rw0'
      block01[o,(i,b)]   = Rw2' * up_b
      block10[(o,a),i]   = Rw1' * up_a
      blk48[(o,a),(i,b)] = Rw5' * (up_a up_b - d_ab/3) + d_ab Rw3' +- Rw4' up_f
    where up = unit(r) in Y1's (y,z,x) component order.
  - The 20 tiles run as a 6-group software pipeline (GROUPS in _run_body) so
    the 20 x 2 MB output-store DMA stream — the memory-roofline resource —
    starts after one short group's latency and then never starves.
  - All matmuls use float32r (1 PE pass instead of 4 for fp32).
  - One ACT table set for the whole kernel (no 1.28 us reloads): sqrt via
    exp(0.5*ln(x)) + one Newton step, silu via exp + DVE reciprocal
    (ln/exp/square/copy all live in the natural_log_exp set; enforced via
    _force_single_act_set).
  - Per tile: 3 PE matmuls -> Rw' in PSUM; consumers read PSUM directly
    (SBUF bandwidth is contended by the store DMA); ACT stages rw1-3 +
    tmp_f = Rw4'*up_f and copies block00; GPSIMD (otherwise idle) builds
    block01/block10; DVE does the 9 scalar_tensor_tensor blocks of blk48;
    one 2 MB store DMA per tile on the sync DGE queue.
Measured (test.py, median paired marginal over on-device For_i loops of
8 vs 1024 iterations): ~146 us/exec, vs ~111 us for the pure 20 x 2 MB
store-DMA stream alone (377 GB/s/core) and ~3.7 us For_i barrier
(baseline metric was ~130 ms of network wall-clock).  Per-tile pools
rotate at depth outp=6 / small=5 / rwB ~3 (shared pool) — recycle-depth
increases were measured wins until saturation at ~146 us, and pool
depths had to be re-tuned after each pipeline restructure (outp=5 was
a loss under the pre-split-store pipeline and a 5 us win after it).  Each tile stores
in two chunks (rows 0-15 after the early ACT/GPSIMD blocks, rows 16-63
after the DVE STT tail) so the store DMA never idles on whole-tile
completion; the radial PSUM is split by consumer lifetime (rwA blocks
0-3 early/ACT, rwB blocks 4-5 held by the STT tail) so double-buffering
covers the long-lived half without stalling tile t+2's matmuls.
HW A/B notes:
125-partition stores fall off the DMA fast path (2.4x); fp16 output
halves DMA but loses more to strided 2-byte engine writes; under store
contention DVE reads PSUM faster than SBUF; multi-queue or paired
stores hurt; in-place diag fixups on g2 serialized the STT consumers
(now folded into W2 host-side: rw3' = rw3 - rw5/3).
"""

import numpy as np

import concourse.bass as bass
import concourse.bacc as bacc
import concourse.tile as tile
from concourse import mybir
from concourse.bass_utils import run_bass_kernel_spmd

MUL = 16
NUM_EDGES = 20000
NUM_BASIS = 64
HIDDEN = 128
R_MAX = 3.0
W = R_MAX / NUM_BASIS          # gaussian width
N_CORES = 8
E_CORE = 2500                  # real edges per core
E_PAD = 2560                   # padded edges per core (20 tiles x 128)
T = 20                         # tiles per core
P = 128                        # partitions (edges per tile)
F32 = mybir.dt.float32
F32R = mybir.dt.float32r
F16 = mybir.dt.float16

# factors folded into W2/b2 path blocks
_PATH_SCALE = np.array([
    1.0 / np.sqrt(32.0),                  # p0 block00
    np.sqrt(3.0) / 8.0,                   # p1 block10 (scalar up_a)
    1.0 / np.sqrt(32.0),                  # p2 block01 (scalar up_b)
    1.0 / 8.0,                            # p3 diag additive
    np.sqrt(3.0) / (8.0 * np.sqrt(2.0)),  # p4 offdiag +-up_f
    3.0 / (8.0 * np.sqrt(2.0)),           # p5 P2[a,b]
], dtype=np.float64)

# offdiag (a,b) -> (f, sign) from eps[a,b,f] (Y1 component order)
_OFFDIAG = [((0, 1), 2, +1), ((1, 0), 2, -1),
            ((1, 2), 0, +1), ((2, 1), 0, -1),
            ((2, 0), 1, +1), ((0, 2), 1, -1)]

from contextlib import contextmanager


@contextmanager
def _force_single_act_set(needed):
    """Steer the act-table-load pass to ONE table set covering `needed`.

    The pass maps each activation func to a set greedily, which bounces
    between sets (1.28 us reload each).  If a single set covers every
    func we use (natural_log_exp has ln+exp+square+copy+identity), blank
    the others so the pass has no choice; set IDs keep their positions so
    walrus still loads the right table.  Falls back to the full tables if
    no single set covers `needed`.
    """
    orig = bacc.get_activation_tables

    def patched(arch):
        tabs = orig(arch)
        for name, funcs in tabs.items():
            if needed <= funcs:
                return {n: (f if n == name else set())
                        for n, f in tabs.items()}
        return tabs

    bacc.get_activation_tables = patched
    try:
        yield
    finally:
        bacc.get_activation_tables = orig


def build_bass(include_b2: bool, reps: int = 1, include_b1: bool = False):
    nc = bacc.Bacc()
    r_in = nc.dram_tensor("r_in", [P, T * 3], F32, kind="ExternalInput")
    w1_in = nc.dram_tensor("w1_in", [NUM_BASIS, HIDDEN], F32R, kind="ExternalInput")
    b1_in = nc.dram_tensor("b1_in", [HIDDEN, 1], F32, kind="ExternalInput")
    w2_in = nc.dram_tensor("w2_in", [HIDDEN, 1536], F32R, kind="ExternalInput")
    if include_b2:
        b2_in = nc.dram_tensor("b2_in", [1, 1536], F32, kind="ExternalInput")
        ones_in = nc.dram_tensor("ones_in", [1, P], F32, kind="ExternalInput")
    cent_in = nc.dram_tensor("cent_in", [P, NUM_BASIS], F32, kind="ExternalInput")
    ident_in = nc.dram_tensor("ident_in", [P, P], F32, kind="ExternalInput")
    out_d = nc.dram_tensor("out_d", [E_PAD, 4096], F32, kind="ExternalOutput")
    # out row (edge) = p*T + t
    out_v = out_d[:, :].rearrange("(p t) n -> p t n", p=P)

    with tile.TileContext(nc) as tc:
        with (
            tc.tile_pool(name="consts", bufs=1) as consts,
            tc.tile_pool(name="geom", bufs=2) as geom,
            tc.tile_pool(name="feat", bufs=2) as feat,
            tc.tile_pool(name="hp_psp", bufs=1, space="PSUM") as hp_psp,
            tc.tile_pool(name="rwa_psp", bufs=2, space="PSUM") as rwa_psp,
            # transpose staging (1 short-lived tile/group) and rwB (the
            # long-lived STT operand, 4 tiles/group) share one 3-deep
            # 1-bank pool: rwB gets ~3-tile recycle distance at the same
            # total PSUM bank count
            tc.tile_pool(name="srw_psp", bufs=3, space="PSUM") as srw_psp,
            tc.tile_pool(name="outp", bufs=6) as outp,
            tc.tile_pool(name="small", bufs=5) as small,
        ):
            # ---- const loads (big w2 last: it gates nothing until the
            # first radial matmul ~20 us in, but hogs the DMA queue) ----
            cent_sb = consts.tile([P, NUM_BASIS], F32)
            nc.sync.dma_start(out=cent_sb, in_=cent_in[:, :])
            ident_sb = consts.tile([P, P], F32)
            nc.sync.dma_start(out=ident_sb, in_=ident_in[:, :])
            w1_sb = consts.tile([NUM_BASIS, HIDDEN], F32R)
            nc.sync.dma_start(out=w1_sb, in_=w1_in[:, :])
            b1_sb = consts.tile([HIDDEN, 1], F32)
            nc.sync.dma_start(out=b1_sb, in_=b1_in[:, :])
            # w2 on the ACT DGE queue so it doesn't serialize ahead of the
            # r-vector load on the sync queue
            w2_sb = consts.tile([HIDDEN, 1536], F32R)
            nc.scalar.dma_start(out=w2_sb, in_=w2_in[:, :])
            if include_b2:
                b2_sb = consts.tile([1, 1536], F32)
                nc.sync.dma_start(out=b2_sb, in_=b2_in[:, :])
                ones_sb = consts.tile([1, P], F32)
                nc.sync.dma_start(out=ones_sb, in_=ones_in[:, :])

            def _body():
                _run_body(nc, include_b2, include_b1, locals_ns)

            locals_ns = dict(
                r_in=r_in, out_v=out_v, w1_sb=w1_sb, b1_sb=b1_sb, w2_sb=w2_sb,
                b2_sb=b2_sb if include_b2 else None,
                ones_sb=ones_sb if include_b2 else None,
                cent_sb=cent_sb, ident_sb=ident_sb,
                geom=geom, feat=feat, srw_psp=srw_psp, hp_psp=hp_psp,
                rwa_psp=rwa_psp, outp=outp, small=small)
            if reps > 1:
                with tc.For_i(0, reps):
                    _body()
            else:
                _body()
    A = mybir.ActivationFunctionType
    needed = {A.Ln, A.Exp, A.Square, A.Copy, A.Identity}
    if include_b1:
        needed.add(A.Silu)
    with _force_single_act_set(needed):
        nc.compile()
    return nc


def _run_body(nc, include_b2, include_b1, ns):
    """One full kernel execution, pipelined in 5 groups of 4 tiles.

    Group g runs geometry -> basis -> transpose -> hidden -> radial/expand ->
    store for tiles 4g..4g+3; its prologue overlaps group g-1's expansion and
    store, so the output DMA stream (the roofline resource: 20 x 2 MB f32)
    starts after only one group's latency and then never starves.
    """
    r_in = ns["r_in"]; out_v = ns["out_v"]
    w1_sb = ns["w1_sb"]; b1_sb = ns["b1_sb"]; w2_sb = ns["w2_sb"]
    b2_sb = ns["b2_sb"]; ones_sb = ns["ones_sb"]
    cent_sb = ns["cent_sb"]; ident_sb = ns["ident_sb"]
    geom = ns["geom"]; feat = ns["feat"]; srw_psp = ns["srw_psp"]
    hp_psp = ns["hp_psp"]; rwa_psp = ns["rwa_psp"]; outp = ns["outp"]
    small = ns["small"]
    w1_r = w1_sb
    w2_r = w2_sb
    GROUPS = [3, 3, 4, 4, 4, 2]   # tiles per group (sum = T)

    # one wide load of all edge vectors (tiny: 240 B/partition)
    r_all = geom.tile([P, T, 3], F32)
    nc.sync.dma_start(out=r_all,
                      in_=r_in[:, :].rearrange("p (t c) -> p t c", c=3))

    t0 = 0
    for g, G in enumerate(GROUPS):
        rg = r_all[:, t0:t0 + G, :]

        # ---- geometry for this group ----
        r2d = geom.tile([P, G, 3], F32, name=f"r2d{g}", tag="r2d")
        nc.vector.tensor_mul(r2d, rg, rg)
        r2 = geom.tile([P, G], F32, name=f"r2{g}", tag="r2")
        nc.vector.reduce_sum(r2, r2d, axis=mybir.AxisListType.X)

        # s0 = sqrt(r2) via exp(0.5*ln(r2)) — Ln and Exp share one ACT
        # table set (natural_log_exp), so the whole kernel runs without
        # any LoadActFuncSet swaps (1.28 us each).  is0 = 1/s0 serves
        # BOTH as the direction normalizer (table precision is fine for
        # unit vectors) and as the Newton-step divisor for the radius
        # feeding the gaussian basis: 2*radii = s0 + r2*is0.
        lr = geom.tile([P, G], F32, name=f"lr{g}", tag="lr")
        nc.scalar.activation(lr, r2, mybir.ActivationFunctionType.Ln)
        s0g = geom.tile([P, G], F32, name=f"s0g{g}", tag="s0g")
        nc.scalar.activation(s0g, lr, mybir.ActivationFunctionType.Exp,
                             scale=0.5)
        is0 = geom.tile([P, G], F32, name=f"is0{g}", tag="is0")
        nc.vector.reciprocal(is0, s0g)

        # up = unit(r); r is already host-permuted to Y1's (y,z,x) order
        up_g = geom.tile([P, G, 3], F32, name=f"up{g}", tag="up")
        nc.vector.tensor_mul(up_g, rg,
                             is0[:, :, None].broadcast_to([P, G, 3]))

        # g2[a,b] = up_a*up_b (the diagonal's -1/3 is folded into W2)
        g2_g = geom.tile([P, G, 3, 3], F32, name=f"g2{g}", tag="g2")
        nc.vector.tensor_mul(
            g2_g,
            up_g[:, :, :, None].broadcast_to([P, G, 3, 3]),
            up_g[:, :, None, :].broadcast_to([P, G, 3, 3]))

        # rwb2 = 2*radii (Newton refined); the 0.5/W scale folds into
        # the basis subtraction below via an immediate-scalar STT
        q_t = geom.tile([P, G], F32, name=f"q{g}", tag="q")
        nc.vector.tensor_mul(q_t, r2, is0)
        rwb = geom.tile([P, G], F32, name=f"rwb{g}", tag="rwb")
        nc.vector.tensor_add(rwb, q_t, s0g)

        # ---- gaussian basis (exp set) ----
        basis_g = feat.tile([P, G, NUM_BASIS], F32, name=f"basis{g}",
                            tag="basis")
        nc.vector.scalar_tensor_tensor(
            basis_g,
            rwb[:, :, None].broadcast_to([P, G, NUM_BASIS]), 0.5 / W,
            cent_sb[:, None, :].broadcast_to([P, G, NUM_BASIS]),
            op0=mybir.AluOpType.mult, op1=mybir.AluOpType.subtract)
        nc.scalar.activation(basis_g, basis_g,
                             mybir.ActivationFunctionType.Square)
        nc.scalar.activation(basis_g, basis_g,
                             mybir.ActivationFunctionType.Exp, scale=-1.0)

        # ---- transpose (PE) + hidden layer (silu set) ----
        bt_ps = srw_psp.tile([NUM_BASIS, G * P], F32, name=f"bt_ps{g}",
                             tag="srw")
        for j in range(G):
            nc.tensor.transpose(bt_ps[:, j * P:(j + 1) * P],
                                basis_g[:, j, :], ident_sb)
        basisT = feat.tile([NUM_BASIS, G * P], F32R, name=f"basisT{g}",
                           tag="basisT")
        nc.scalar.copy(basisT, bt_ps)

        hp_ps = hp_psp.tile([HIDDEN, G * P], F32, name=f"hp_ps{g}",
                            tag="hp_ps")
        nc.tensor.matmul(hp_ps, w1_r, basisT,
                         start=True, stop=True)
        h_T = feat.tile([HIDDEN, G * P], F32R, name=f"h_T{g}", tag="h_T")
        if include_b1:
            # general path: table silu (costs a table-set swap per group)
            nc.scalar.activation(h_T, hp_ps,
                                 mybir.ActivationFunctionType.Silu,
                                 bias=b1_sb)
        else:
            # silu(x) = x / (1 + exp(-x)) with exp from the shared table
            # set; division on DVE.  b1 == 0 so no bias is needed.
            eh = feat.tile([HIDDEN, G * P], F32, name=f"eh{g}", tag="eh")
            nc.scalar.activation(eh, hp_ps,
                                 mybir.ActivationFunctionType.Exp,
                                 scale=-1.0)
            nc.vector.tensor_scalar_add(eh, eh, 1.0)
            nc.vector.reciprocal(eh, eh)
            nc.vector.tensor_mul(h_T, hp_ps, eh)
        h_Tr = h_T

        # ---- per-tile radial weights + expansion + store ----
        for j in range(G):
            t = t0 + j
            # radial weights split over two PSUM pools by consumer
            # lifetime: rwA (blocks 0-3) is read only by early ACT ops and
            # recycles fast; rwB (blocks 4-5) is held until the DVE STT
            # tail, so its small 1-bank tile is what double-buffers across
            # the tile pipeline instead of the whole 3-bank tensor.
            rwa_ps = rwa_psp.tile([P, 1024], F32, name=f"rwa{t}", tag="rwa")
            rwb_ps = srw_psp.tile([P, 512], F32, name=f"rwb{t}", tag="srw")
            hT_t = h_Tr[:, j * P:(j + 1) * P]
            for k in range(3):
                dst = (rwa_ps[:, k * 512:(k + 1) * 512] if k < 2
                       else rwb_ps)
                nc.tensor.matmul(dst, hT_t,
                                 w2_r[:, k * 512:(k + 1) * 512],
                                 start=True, stop=not include_b2)
                if include_b2:
                    nc.tensor.matmul(dst, ones_sb,
                                     b2_sb[:, k * 512:(k + 1) * 512],
                                     start=False, stop=True)

            # consumers read the radial weights straight from PSUM to
            # spare SBUF write/read bandwidth (it contends with the
            # 2 MB/tile output-store DMA reads); only STT in1 and the
            # GPSIMD operands need an SBUF stage
            rwA = rwa_ps.rearrange("p (q o i) -> p q o i", q=4, o=16)
            rwB = rwb_ps.rearrange("p (q o i) -> p q o i", q=2, o=16)

            ot = outp.tile([P, 4096], F32, name=f"ot{t}", tag="ot")
            otm = ot.rearrange("p (r c) -> p r c", r=64)
            b01 = otm[:, 0:16, 16:64].rearrange("p o (i b) -> p o i b", b=3)
            b10 = otm[:, 16:64, 0:16].rearrange("p (o a) i -> p o a i", a=3)
            b48 = otm[:, 16:64, 16:64].rearrange(
                "p (o a) (i b) -> p o a i b", a=3, b=3)

            up_t = [up_g[:, j, f:f + 1] for f in range(3)]

            # tmp FIRST on ACT: the 6 offdiag STTs (2/3 of the DVE tail
            # that gates the late store chunk) need only tmp + rwB, so
            # producing tmp before the stage/block00 copies lets the DVE
            # tail start ~1 us earlier each tile
            tmp = small.tile([P, 3, 16, 16], F32, name=f"tmp{t}", tag="tmp")
            for f in range(3):
                nc.scalar.activation(tmp[:, f], rwB[:, 0],
                                     mybir.ActivationFunctionType.Copy,
                                     scale=up_t[f])
            rw_sb = small.tile([P, 3, 16, 16], F32, name=f"rw_sb{t}",
                               tag="rw_sb")
            nc.scalar.copy(rw_sb, rwA[:, 1:4])
            # block00 = Rw0' (ACT; DVE is the stream-limiting engine)
            nc.scalar.copy(otm[:, 0:16, 0:16], rwA[:, 0])
            # block01[o,(i,b)] = Rw2' * up_b (one broadcast TT on GPSIMD)
            nc.gpsimd.tensor_mul(
                b01,
                rw_sb[:, 1][:, :, :, None].broadcast_to([P, 16, 16, 3]),
                up_g[:, j, None, None, :].broadcast_to([P, 16, 16, 3]))
            # rows 0..15 (block00 + block01) are done: ship them while the
            # DVE STT tail still builds rows 16..63 — keeps the store DMA
            # fed instead of idling on whole-tile completion
            nc.sync.dma_start(out=out_v[:, t, 0:1024], in_=ot[:, 0:1024])
            # block10[(o,a),i] = Rw1' * up_a (one broadcast TT on GPSIMD)
            nc.gpsimd.tensor_mul(
                b10,
                rw_sb[:, 0][:, :, None, :].broadcast_to([P, 16, 3, 16]),
                up_g[:, j, None, :, None].broadcast_to([P, 16, 3, 16]))
            # 48-block offdiag first (needs only tmp+rwB): Rw5'*P2ab +- tmp_f
            for (a, b), f, sgn in _OFFDIAG:
                nc.vector.scalar_tensor_tensor(
                    b48[:, :, a, :, b], rwB[:, 1], g2_g[:, j, a, b:b + 1],
                    tmp[:, f], op0=mybir.AluOpType.mult,
                    op1=(mybir.AluOpType.add if sgn > 0
                         else mybir.AluOpType.subtract))
            # 48-block diag: Rw5'*P2aa + Rw3'
            for a in range(3):
                nc.vector.scalar_tensor_tensor(
                    b48[:, :, a, :, a], rwB[:, 1], g2_g[:, j, a, a:a + 1],
                    rw_sb[:, 2], op0=mybir.AluOpType.mult,
                    op1=mybir.AluOpType.add)

            nc.sync.dma_start(out=out_v[:, t, 1024:4096],
                              in_=ot[:, 1024:4096])
        t0 += G

_NC_CACHE = {}


def _get_nc(include_b2: bool, reps: int = 1, include_b1: bool = False):
    key = (include_b2, reps, include_b1)
    if key not in _NC_CACHE:
        _NC_CACHE[key] = build_bass(include_b2, reps, include_b1)
    return _NC_CACHE[key]


def prep_inputs(r, W1, b1, W2, b2):
    """Host-side prep: pad + (p,t)-permute r shards, prescale W2/b2, consts."""
    r = np.ascontiguousarray(np.asarray(r, np.float32))
    W2s = (np.asarray(W2, np.float64).reshape(HIDDEN, 6, 256)
           * _PATH_SCALE[None, :, None])
    b2s = np.asarray(b2, np.float64).reshape(6, 256) * _PATH_SCALE[:, None]
    # fold the P2-diagonal "-1/3" into the additive radial block:
    # rw5*(up_a^2 - 1/3) + rw3  ==  rw5*up_a^2 + (rw3 - rw5/3)
    W2s[:, 3, :] -= W2s[:, 5, :] / 3.0
    b2s[3, :] -= b2s[5, :] / 3.0
    W2s = W2s.reshape(HIDDEN, 1536).astype(np.float32)
    b2s = b2s.reshape(1, 1536).astype(np.float32)
    centers = np.linspace(0.0, R_MAX, NUM_BASIS).astype(np.float32)
    cent_rep = np.tile((centers / np.float32(W))[None, :], (P, 1))
    ident = np.eye(P, dtype=np.float32)
    ones = np.ones((1, P), np.float32)
    b1c = np.asarray(b1, np.float32).reshape(HIDDEN, 1)
    w1 = np.ascontiguousarray(np.asarray(W1, np.float32))

    in_maps = []
    for c in range(N_CORES):
        shard = r[c * E_CORE:(c + 1) * E_CORE]
        pad = np.tile(np.array([[1.0, 0.0, 0.0]], np.float32),
                      (E_PAD - shard.shape[0], 1))
        shard = np.concatenate([shard, pad], 0)      # [2560, 3], row = p*T+t
        shard = shard[:, [1, 2, 0]]                  # (y,z,x): Y1 order
        in_maps.append({
            "r_in": np.ascontiguousarray(shard.reshape(P, T * 3)),
            "w1_in": w1, "b1_in": b1c, "w2_in": W2s, "b2_in": b2s,
            "cent_in": cent_rep, "ident_in": ident, "ones_in": ones,
        })
    return in_maps


def _kernel2(wl0, wl1):
    """Reference fallback for |r| == 0 edges (computed host-side)."""
    k2 = np.zeros((64, 64), np.float32)
    k2[:16, :16] = np.asarray(wl0, np.float32) / np.sqrt(np.float32(MUL))
    k2[16:, 16:] = np.kron(np.asarray(wl1, np.float32),
                           np.eye(3, dtype=np.float32)) / np.sqrt(np.float32(MUL))
    return k2


def _make_jit(nc):
    """jit-compiled 8-core SPMD dispatcher for one compiled bass program."""
    import jax
    from jax.sharding import Mesh, PartitionSpec
    try:
        from jax.experimental.shard_map import shard_map
    except ImportError:
        from jax.shard_map import shard_map  # newer jax
    from concourse import bass2jax as b2j

    b2j.install_neuronx_cc_hook()
    part_name = nc.partition_id_tensor.name if nc.partition_id_tensor else None
    in_names, out_names, out_avals = [], [], []
    for alloc in nc.m.functions[0].allocations:
        if not isinstance(alloc, mybir.MemoryLocationSet):
            continue
        nm = alloc.memorylocations[0].name
        if alloc.kind == "ExternalInput":
            if nm != part_name:
                in_names.append(nm)
        elif alloc.kind == "ExternalOutput":
            out_names.append(nm)
            out_avals.append(jax.core.ShapedArray(
                tuple(alloc.tensor_shape), mybir.dt.np(alloc.dtype)))
    n_params = len(in_names)
    all_in = list(in_names + out_names)
    if part_name is not None:
        all_in.append(part_name)
    n_outs = len(out_names)

    def _body(*args):
        operands = list(args)
        if part_name is not None:
            operands.append(b2j.partition_id_tensor())
        outs = b2j._bass_exec_p.bind(
            *operands, out_avals=tuple(out_avals), in_names=tuple(all_in),
            out_names=tuple(out_names), lowering_input_output_aliases=(),
            sim_require_finite=True, sim_require_nnan=True, nc=nc)
        return tuple(outs)

    devices = jax.devices()[:N_CORES]
    mesh = Mesh(np.asarray(devices), ("core",))
    donate = tuple(range(n_params, n_params + n_outs))
    f = jax.jit(
        shard_map(_body, mesh=mesh,
                  in_specs=(PartitionSpec("core"),) * (n_params + n_outs),
                  out_specs=(PartitionSpec("core"),) * n_outs,
                  check_rep=False),
        donate_argnums=donate, keep_unused=True)
    return f, in_names, mesh


def bench(inputs, reps, krep1=8, krep2=1024):
    """Dev-only: measure per-execution device time of the kernel.

    A single dispatch through the axon network tunnel has a fixed ~80 ms
    RPC floor (with multi-ms jitter) that is three orders of magnitude
    above the kernel itself, so single-shot wall time measures the
    network, not the hardware.  We therefore time the SAME kernel body
    wrapped in an on-device hardware loop (tc.For_i; krep1 vs krep2
    iterations per dispatch) and report the marginal cost per iteration:
    (wall(krep2) - wall(krep1)) / (krep2 - krep1).  The large iteration
    spread divides the dispatch jitter by ~500.  Inputs are device-
    resident; each timed dispatch re-runs the full computation (geometry,
    radial MLP, tensor-product expansion, HBM store) krep times.  The
    For_i all-engine barrier prevents cross-iteration overlap, so the
    marginal cost is a faithful (slightly conservative) single-shot
    execution time.

    Returns (per_exec_seconds, diagnostics dict).
    """
    import time
    import jax
    from jax.sharding import NamedSharding, PartitionSpec

    r = np.asarray(inputs["r"], np.float32)
    include_b2 = bool(np.any(np.asarray(inputs["b2"]) != 0.0))
    in_maps = prep_inputs(r, inputs["W1"], inputs["b1"], inputs["W2"],
                          inputs["b2"])
    if not include_b2:
        for m in in_maps:
            m.pop("b2_in")
            m.pop("ones_in")

    include_b1 = bool(np.any(np.asarray(inputs["b1"]) != 0.0))
    nc1 = _get_nc(include_b2, krep1, include_b1)
    nck = _get_nc(include_b2, krep2, include_b1)
    f1, in_names, mesh = _make_jit(nc1)
    fk, in_names_k, _ = _make_jit(nck)
    assert in_names == in_names_k
    sh = NamedSharding(mesh, PartitionSpec("core"))
    concat_in = [np.concatenate([np.asarray(m[k]) for m in in_maps], 0)
                 for k in in_names]
    dev_in = [jax.device_put(a, sh) for a in concat_in]
    jax.block_until_ready(dev_in)

    def run_once(f, outs):
        t0 = time.perf_counter()
        outs = list(f(*dev_in, *outs))  # donated outputs recycled
        jax.block_until_ready(outs)
        return time.perf_counter() - t0, outs

    # warm both compiled paths
    outs = [np.zeros((N_CORES * E_PAD, 4096), np.float32)]
    _, outs = run_once(f1, outs)
    _, outs = run_once(fk, outs)
    # interleave the two loop lengths so each paired difference sees the
    # same network/dispatch conditions; the dispatch floor drifts by
    # multiple ms between runs, so unpaired mins are biased
    ts1, tsk = [], []
    for _ in range(reps):
        t1, outs = run_once(f1, outs)
        tk, outs = run_once(fk, outs)
        ts1.append(t1)
        tsk.append(tk)
    diffs = sorted(tk - t1 for t1, tk in zip(ts1, tsk))
    med = diffs[len(diffs) // 2] if len(diffs) % 2 else 0.5 * (
        diffs[len(diffs) // 2 - 1] + diffs[len(diffs) // 2])
    per_exec = med / (krep2 - krep1)
    # guard: the looped NEFF must still produce the correct output
    looped = np.asarray(outs[0]).astype(np.float32)
    looped = looped.reshape(N_CORES, E_PAD, 4096)[:, :E_CORE]
    looped = looped.reshape(NUM_EDGES, 64, 64)
    diag = {
        "wall_k1_ms": min(ts1) * 1e3,
        "wall_k2_ms": min(tsk) * 1e3,
        "krep1": krep1,
        "krep2": krep2,
        "looped_output": looped,
    }
    return per_exec, diag


def kernel(r, W1, b1, W2, b2, wl0, wl1, **_):
    r = np.asarray(r, np.float32)
    include_b2 = bool(np.any(np.asarray(b2) != 0.0))
    include_b1 = bool(np.any(np.asarray(b1) != 0.0))
    nc = _get_nc(include_b2, 1, include_b1)
    in_maps = prep_inputs(r, W1, b1, W2, b2)
    if not include_b2:
        for m in in_maps:
            m.pop("b2_in")
            m.pop("ones_in")
    res = run_bass_kernel_spmd(nc, in_maps, core_ids=list(range(N_CORES)))
    full = np.concatenate(
        [res.results[c]["out_d"][:E_CORE] for c in range(N_CORES)],
        0).astype(np.float32).reshape(NUM_EDGES, 64, 64)
    zero_rows = np.flatnonzero(np.linalg.norm(r, axis=1) == 0.0)
    if zero_rows.size:
        full = full.copy()
        full[zero_rows] = _kernel2(wl0, wl1)[None]
    return full

